# revision 2
# baseline (speedup 1.0000x reference)
"""Channel-attention (mean + top-4 sum -> shared MLP -> sigmoid gate -> scale)
distributed over 8 TRN2 NeuronCores.

Layout: (b, c) on the 128 SBUF partitions, spatial on the free axis.
Sharding: the D spatial axis is split 8 ways (one contiguous chunk per core).
Per core: stream the 64 MiB local shard once, computing the per-(b,c) running
sum on ScalarE (activation accum_out) and the per-tile top-8 on VectorE
(InstMax) in the same pass; AllGather the tiny [128, 9] per-core stats; merge
(exact top-4 = top-4 of the 8 gathered descending top-8 lists); run the tiny
MLP on TensorE with block-diagonal weights (both batches in one matmul, both
branch stats as the 2-wide moving operand); then stream the shard a second
time multiplying by the per-partition sigmoid gate.
"""

import os
import sys

import numpy as np


def _ensure_imports():
    try:
        import concourse.bass  # noqa: F401
        return
    except ImportError:
        pass
    for p in ("/root/.axon_site/_ro/trn_rl_repo", "/opt/trn_rl_repo"):
        if os.path.isdir(p) and p not in sys.path:
            sys.path.append(p)
    import concourse.bass  # noqa: F401


_ensure_imports()

from concourse import bacc, mybir, tile  # noqa: E402
from concourse.bass_utils import run_bass_kernel_spmd  # noqa: E402

B, C, D, H, W = 2, 64, 64, 128, 128
NCORES = 8
P = B * C                # 128 partitions = (b, c)
DSH = D // NCORES        # 8 D-planes per core
F = DSH * H * W          # 131072 free elements per partition per core
FT = 8192                # free-dim tile size
NT = F // FT             # 16 tiles per pass
TOPK = 4
F32 = mybir.dt.float32

_CACHE = {}


def _build():
    nc = bacc.Bacc(
        "TRN2", target_bir_lowering=False, debug=False, num_devices=NCORES
    )
    x_in = nc.declare_dram_parameter("x", [P, F], F32, isOutput=False)
    w1_in = nc.declare_dram_parameter("W1", [C // 2, C], F32, isOutput=False)
    b1_in = nc.declare_dram_parameter("b1", [1, C // 2], F32, isOutput=False)
    w2_in = nc.declare_dram_parameter("W2", [C, C // 2], F32, isOutput=False)
    b2_in = nc.declare_dram_parameter("b2", [1, C], F32, isOutput=False)
    out_x = nc.declare_dram_parameter("out", [P, F], F32, isOutput=True)
    gate_out = nc.declare_dram_parameter("gate", [P, 1], F32, isOutput=True)

    cc_in = nc.dram_tensor("cc_in", [P, 9], F32)
    cc_out = nc.dram_tensor("cc_out", [NCORES, P, 9], F32, addr_space="Shared")

    AFT = mybir.ActivationFunctionType
    AX = mybir.AxisListType
    rg = [list(range(NCORES))]
    HC = C // 2

    with tile.TileContext(nc) as tc:
        with (
            tc.tile_pool(name="big", bufs=5) as big,
            tc.tile_pool(name="small", bufs=1) as small,
            tc.tile_pool(name="ps", bufs=1, space="PSUM") as ps,
        ):
            # Block-diagonal MLP weights: both batches share the MLP, so one
            # [128,64] stationary computes fc1 for b=0 and b=1 at once.
            w1bd = small.tile([P, C], F32)       # lhsT: [k=bc, m=b*32+j]
            w2bd = small.tile([C, P], F32)       # lhsT: [k=b*32+j, m=bc]
            b1bd = small.tile([C, 1], F32)
            b2x2 = small.tile([P, 1], F32)       # 2*b2 (b2 appears in both fc branches)
            nc.vector.memset(w1bd[:], 0.0)
            nc.vector.memset(w2bd[:], 0.0)
            for b in range(B):
                nc.sync.dma_start(
                    out=w1bd[b * C:(b + 1) * C, b * HC:(b + 1) * HC],
                    in_=w1_in[:].rearrange("j c -> c j"),
                )
                nc.sync.dma_start(
                    out=w2bd[b * HC:(b + 1) * HC, b * C:(b + 1) * C],
                    in_=w2_in[:].rearrange("c j -> j c"),
                )
                nc.sync.dma_start(
                    out=b1bd[b * HC:(b + 1) * HC, :],
                    in_=b1_in[:].rearrange("a j -> j a"),
                )
                nc.sync.dma_start(
                    out=b2x2[b * C:(b + 1) * C, :],
                    in_=b2_in[:].rearrange("a c -> c a"),
                )
            nc.scalar.mul(b2x2[:], b2x2[:], 2.0)

            # Pass 1: stream the shard; ScalarE accumulates the per-tile sum
            # while VectorE extracts the per-tile top-8.
            sum_slots = small.tile([P, NT], F32)
            top8_slots = small.tile([P, NT * 8], F32)
            for i in range(NT):
                xt = big.tile([P, FT], F32, tag="xt")
                nc.sync.dma_start(out=xt[:], in_=x_in[:, i * FT:(i + 1) * FT])
                nc.scalar.activation(
                    xt[:], xt[:], AFT.Copy, accum_out=sum_slots[:, i:i + 1]
                )
                nc.vector.max(out=top8_slots[:, i * 8:(i + 1) * 8], in_=xt[:])

            # Local merge -> [top8 | sum] = [128, 9]
            stats9 = small.tile([P, 9], F32)
            nc.vector.max(out=stats9[:, 0:8], in_=top8_slots[:])
            nc.vector.reduce_sum(out=stats9[:, 8:9], in_=sum_slots[:], axis=AX.X)

            # Cross-core merge: tiny AllGather (8 x 4.5 KiB)
            nc.sync.dma_start(out=cc_in[:], in_=stats9[:])
            nc.gpsimd.collective_compute(
                "AllGather",
                mybir.AluOpType.bypass,
                replica_groups=rg,
                ins=[cc_in[:].opt()],
                outs=[cc_out[:].opt()],
            )
            gath = small.tile([P, NCORES * 9], F32)
            nc.sync.dma_start(
                out=gath[:].rearrange("p (r e) -> p r e", e=9),
                in_=cc_out[:].rearrange("r p e -> p r e"),
            )

            # Global merge: exact top-4 = first 4 of top-8 of the gathered
            # per-core descending top-8 lists; mean from the summed sums.
            g_top8 = small.tile([P, 8], F32)
            gv = gath[:].rearrange("p (r e) -> p r e", e=9)
            nc.vector.max(out=g_top8[:], in_=gv[:, :, 0:8])
            stats2 = small.tile([P, 2], F32)
            gs = gath[:].rearrange("p (r e) -> p e r", e=9)
            nc.vector.reduce_sum(out=stats2[:, 0:1], in_=gs[:, 8:9, :], axis=AX.X)
            nc.scalar.mul(stats2[:, 0:1], stats2[:, 0:1], 1.0 / (D * H * W))
            nc.vector.reduce_sum(out=stats2[:, 1:2], in_=g_top8[:, 0:4], axis=AX.X)

            # MLP: h = relu(W1 @ stats + b1); s = W2 @ h; gate = sigmoid(s0 + s1 + 2*b2)
            h_ps = ps.tile([C, B], F32)
            nc.tensor.matmul(h_ps[:], w1bd[:], stats2[:], start=True, stop=True)
            h_sb = small.tile([C, B], F32)
            nc.scalar.activation(h_sb[:], h_ps[:], AFT.Relu, bias=b1bd[:], scale=1.0)
            s_ps = ps.tile([P, B], F32)
            nc.tensor.matmul(s_ps[:], w2bd[:], h_sb[:], start=True, stop=True)
            ssum = small.tile([P, 1], F32)
            nc.vector.reduce_sum(out=ssum[:], in_=s_ps[:], axis=AX.X)
            gate_sb = small.tile([P, 1], F32)
            nc.scalar.activation(gate_sb[:], ssum[:], AFT.Sigmoid, bias=b2x2[:], scale=1.0)
            nc.sync.dma_start(out=gate_out[:], in_=gate_sb[:])

            # Pass 2: stream the shard again, scale by the per-partition gate.
            # Alternate ScalarE/VectorE so neither engine is the bottleneck.
            for i in range(NT):
                yt = big.tile([P, FT], F32, tag="xt")
                nc.sync.dma_start(out=yt[:], in_=x_in[:, i * FT:(i + 1) * FT])
                if i % 2 == 0:
                    nc.vector.tensor_scalar_mul(yt[:], yt[:], gate_sb[:])
                else:
                    nc.scalar.mul(yt[:], yt[:], gate_sb[:])
                nc.sync.dma_start(out=out_x[:, i * FT:(i + 1) * FT], in_=yt[:])

    nc.compile()
    return nc


def _get_nc():
    if "nc" not in _CACHE:
        _CACHE["nc"] = _build()
    return _CACHE["nc"]


def _make_in_maps(x, W1, b1, W2, b2):
    x = np.ascontiguousarray(x, dtype=np.float32)
    w1 = np.ascontiguousarray(W1, dtype=np.float32)
    b1r = np.ascontiguousarray(b1, dtype=np.float32).reshape(1, HC_CONST)
    w2 = np.ascontiguousarray(W2, dtype=np.float32)
    b2r = np.ascontiguousarray(b2, dtype=np.float32).reshape(1, C)
    in_maps = []
    for i in range(NCORES):
        shard = np.ascontiguousarray(
            x[:, :, i * DSH:(i + 1) * DSH]
        ).reshape(P, F)
        in_maps.append(
            {"x": shard, "W1": w1, "b1": b1r, "W2": w2, "b2": b2r}
        )
    return in_maps


HC_CONST = C // 2


def run_device(x, W1, b1, W2, b2, trace=False, **kwargs):
    """Run the SPMD kernel; returns (BassKernelResults, assembled outputs)."""
    nc = _get_nc()
    in_maps = _make_in_maps(x, W1, b1, W2, b2)
    res = run_bass_kernel_spmd(
        nc, in_maps, core_ids=list(range(NCORES)), trace=trace, **kwargs
    )
    scaled = np.empty((B, C, D, H, W), dtype=np.float32)
    for i in range(NCORES):
        scaled[:, :, i * DSH:(i + 1) * DSH] = (
            res.results[i]["out"].reshape(B, C, DSH, H, W)
        )
    gate = res.results[0]["gate"].reshape(B, C)
    return res, (scaled, gate)


def kernel(x, W1, b1, W2, b2):
    _, outs = run_device(x, W1, b1, W2, b2, trace=False)
    return outs


# revision 8
# speedup vs baseline: 1.0393x; 1.0393x over previous
"""Channel-attention (mean + top-4 sum -> shared MLP -> sigmoid gate -> scale)
distributed over 8 TRN2 NeuronCores.

Layout: (b, c) on the 128 SBUF partitions, spatial on the free axis.
Sharding: the D spatial axis is split 8 ways (one contiguous chunk per core).
Per core: stream the 64 MiB local shard once, computing the per-(b,c) running
sum on ScalarE (activation accum_out) and the per-tile top-8 on VectorE
(InstMax) in the same pass; AllGather the tiny [128, 9] per-core stats; merge
(exact top-4 = top-4 of the 8 gathered descending top-8 lists); run the tiny
MLP on TensorE with block-diagonal weights (both batches in one matmul, both
branch stats as the 2-wide moving operand); then stream the shard a second
time multiplying by the per-partition sigmoid gate.
"""

import os
import sys

import numpy as np


def _ensure_imports():
    try:
        import concourse.bass  # noqa: F401
        return
    except ImportError:
        pass
    for p in ("/root/.axon_site/_ro/trn_rl_repo", "/opt/trn_rl_repo"):
        if os.path.isdir(p) and p not in sys.path:
            sys.path.append(p)
    import concourse.bass  # noqa: F401


_ensure_imports()

from concourse import bacc, mybir, tile  # noqa: E402
from concourse.bass_utils import run_bass_kernel_spmd  # noqa: E402

B, C, D, H, W = 2, 64, 64, 128, 128
NCORES = 8
P = B * C                # 128 partitions = (b, c)
DSH = D // NCORES        # 8 D-planes per core
F = DSH * H * W          # 131072 free elements per partition per core
FT = 8192                # free-dim tile size
NT = F // FT             # 16 tiles per pass
TOPK = 4
NCACHE = 6               # leading tiles kept resident in SBUF as bf16
F32 = mybir.dt.float32
BF16 = mybir.dt.bfloat16

_CACHE = {}


def _build():
    nc = bacc.Bacc(
        "TRN2", target_bir_lowering=False, debug=False, num_devices=NCORES
    )
    x_in = nc.declare_dram_parameter("x", [P, F], F32, isOutput=False)
    w1_in = nc.declare_dram_parameter("W1", [C // 2, C], F32, isOutput=False)
    b1_in = nc.declare_dram_parameter("b1", [1, C // 2], F32, isOutput=False)
    w2_in = nc.declare_dram_parameter("W2", [C, C // 2], F32, isOutput=False)
    b2_in = nc.declare_dram_parameter("b2", [1, C], F32, isOutput=False)
    out_x = nc.declare_dram_parameter("out", [P, F], F32, isOutput=True)
    gate_out = nc.declare_dram_parameter("gate", [P, 1], F32, isOutput=True)

    cc_in = nc.dram_tensor("cc_in", [P, 9], F32)
    cc_out = nc.dram_tensor("cc_out", [NCORES, P, 9], F32, addr_space="Shared")

    AFT = mybir.ActivationFunctionType
    AX = mybir.AxisListType
    rg = [list(range(NCORES))]
    HC = C // 2

    with tile.TileContext(nc) as tc:
        with (
            tc.tile_pool(name="big", bufs=3) as big,
            tc.tile_pool(name="cache", bufs=1) as cache,
            tc.tile_pool(name="small", bufs=1) as small,
            tc.tile_pool(name="ps", bufs=1, space="PSUM") as ps,
        ):
            # Block-diagonal MLP weights: both batches share the MLP, so one
            # [128,64] stationary computes fc1 for b=0 and b=1 at once.
            w1bd = small.tile([P, C], F32)       # lhsT: [k=bc, m=b*32+j]
            w2bd = small.tile([C, P], F32)       # lhsT: [k=b*32+j, m=bc]
            b1bd = small.tile([C, 1], F32)
            b2x2 = small.tile([P, 1], F32)       # 2*b2 (b2 appears in both fc branches)
            nc.vector.memset(w1bd[:], 0.0)
            nc.vector.memset(w2bd[:], 0.0)
            for b in range(B):
                nc.sync.dma_start(
                    out=w1bd[b * C:(b + 1) * C, b * HC:(b + 1) * HC],
                    in_=w1_in[:].rearrange("j c -> c j"),
                )
                nc.sync.dma_start(
                    out=w2bd[b * HC:(b + 1) * HC, b * C:(b + 1) * C],
                    in_=w2_in[:].rearrange("c j -> j c"),
                )
                nc.sync.dma_start(
                    out=b1bd[b * HC:(b + 1) * HC, :],
                    in_=b1_in[:].rearrange("a j -> j a"),
                )
                nc.sync.dma_start(
                    out=b2x2[b * C:(b + 1) * C, :],
                    in_=b2_in[:].rearrange("a c -> c a"),
                )
            nc.scalar.mul(b2x2[:], b2x2[:], 2.0)

            # Pass 1: stream the shard; ScalarE accumulates the per-tile sum
            # while VectorE extracts the per-tile top-8. The first NCACHE
            # tiles stay resident in SBUF as bf16 (cast fused into the
            # ScalarE sum-copy) so pass 2 can skip their HBM re-read.
            sum_slots = small.tile([P, NT], F32)
            top8_slots = small.tile([P, NT * 8], F32)
            cached = [
                cache.tile([P, FT], BF16, tag=f"cache{i}", name=f"cache{i}")
                for i in range(NCACHE)
            ]
            for i in range(NT):
                xt = big.tile([P, FT], F32, tag="xt")
                nc.sync.dma_start(out=xt[:], in_=x_in[:, i * FT:(i + 1) * FT])
                acc_dst = cached[i][:] if i < NCACHE else xt[:]
                nc.scalar.activation(
                    acc_dst, xt[:], AFT.Copy, accum_out=sum_slots[:, i:i + 1]
                )
                nc.vector.max(out=top8_slots[:, i * 8:(i + 1) * 8], in_=xt[:])

            # Local merge -> [top8 | sum] = [128, 9]
            stats9 = small.tile([P, 9], F32)
            nc.vector.max(out=stats9[:, 0:8], in_=top8_slots[:])
            nc.vector.reduce_sum(out=stats9[:, 8:9], in_=sum_slots[:], axis=AX.X)

            # Cross-core merge: tiny AllGather (8 x 4.5 KiB)
            nc.sync.dma_start(out=cc_in[:], in_=stats9[:])
            nc.gpsimd.collective_compute(
                "AllGather",
                mybir.AluOpType.bypass,
                replica_groups=rg,
                ins=[cc_in[:].opt()],
                outs=[cc_out[:].opt()],
            )
            gath = small.tile([P, NCORES * 9], F32)
            nc.sync.dma_start(
                out=gath[:].rearrange("p (r e) -> p r e", e=9),
                in_=cc_out[:].rearrange("r p e -> p r e"),
            )

            # Global merge: exact top-4 = first 4 of top-8 of the gathered
            # per-core descending top-8 lists; mean from the summed sums.
            g_top8 = small.tile([P, 8], F32)
            gv = gath[:].rearrange("p (r e) -> p r e", e=9)
            nc.vector.max(out=g_top8[:], in_=gv[:, :, 0:8])
            stats2 = small.tile([P, 2], F32)
            gs = gath[:].rearrange("p (r e) -> p e r", e=9)
            nc.vector.reduce_sum(out=stats2[:, 0:1], in_=gs[:, 8:9, :], axis=AX.X)
            nc.scalar.mul(stats2[:, 0:1], stats2[:, 0:1], 1.0 / (D * H * W))
            nc.vector.reduce_sum(out=stats2[:, 1:2], in_=g_top8[:, 0:4], axis=AX.X)

            # MLP: h = relu(W1 @ stats + b1); s = W2 @ h; gate = sigmoid(s0 + s1 + 2*b2)
            h_ps = ps.tile([C, B], F32)
            nc.tensor.matmul(h_ps[:], w1bd[:], stats2[:], start=True, stop=True)
            h_sb = small.tile([C, B], F32)
            nc.scalar.activation(h_sb[:], h_ps[:], AFT.Relu, bias=b1bd[:], scale=1.0)
            s_ps = ps.tile([P, B], F32)
            nc.tensor.matmul(s_ps[:], w2bd[:], h_sb[:], start=True, stop=True)
            ssum = small.tile([P, 1], F32)
            nc.vector.reduce_sum(out=ssum[:], in_=s_ps[:], axis=AX.X)
            gate_sb = small.tile([P, 1], F32)
            nc.scalar.activation(gate_sb[:], ssum[:], AFT.Sigmoid, bias=b2x2[:], scale=1.0)
            nc.scalar.dma_start(out=gate_out[:], in_=gate_sb[:])

            # Pass 2: scale by the per-partition gate. Cached tiles multiply
            # straight out of SBUF (ScalarE casts bf16->f32); the rest
            # re-stream from HBM. Loads go on the Sync HWDGE queue, stores on
            # the Scalar HWDGE queue so the two rings interleave.
            for i in range(NCACHE):
                ot = big.tile([P, FT], F32, tag="xt")
                nc.scalar.mul(ot[:], cached[i][:], gate_sb[:])
                nc.scalar.dma_start(out=out_x[:, i * FT:(i + 1) * FT], in_=ot[:])
            for i in range(NCACHE, NT):
                yt = big.tile([P, FT], F32, tag="xt")
                nc.sync.dma_start(out=yt[:], in_=x_in[:, i * FT:(i + 1) * FT])
                nc.vector.tensor_scalar_mul(yt[:], yt[:], gate_sb[:])
                nc.scalar.dma_start(out=out_x[:, i * FT:(i + 1) * FT], in_=yt[:])

    nc.compile()
    return nc


def _get_nc():
    if "nc" not in _CACHE:
        _CACHE["nc"] = _build()
    return _CACHE["nc"]


def _make_in_maps(x, W1, b1, W2, b2):
    x = np.ascontiguousarray(x, dtype=np.float32)
    w1 = np.ascontiguousarray(W1, dtype=np.float32)
    b1r = np.ascontiguousarray(b1, dtype=np.float32).reshape(1, HC_CONST)
    w2 = np.ascontiguousarray(W2, dtype=np.float32)
    b2r = np.ascontiguousarray(b2, dtype=np.float32).reshape(1, C)
    in_maps = []
    for i in range(NCORES):
        shard = np.ascontiguousarray(
            x[:, :, i * DSH:(i + 1) * DSH]
        ).reshape(P, F)
        in_maps.append(
            {"x": shard, "W1": w1, "b1": b1r, "W2": w2, "b2": b2r}
        )
    return in_maps


HC_CONST = C // 2


def run_device(x, W1, b1, W2, b2, trace=False, **kwargs):
    """Run the SPMD kernel; returns (BassKernelResults, assembled outputs)."""
    nc = _get_nc()
    in_maps = _make_in_maps(x, W1, b1, W2, b2)
    res = run_bass_kernel_spmd(
        nc, in_maps, core_ids=list(range(NCORES)), trace=trace, **kwargs
    )
    scaled = np.empty((B, C, D, H, W), dtype=np.float32)
    for i in range(NCORES):
        scaled[:, :, i * DSH:(i + 1) * DSH] = (
            res.results[i]["out"].reshape(B, C, DSH, H, W)
        )
    gate = res.results[0]["gate"].reshape(B, C)
    return res, (scaled, gate)


def kernel(x, W1, b1, W2, b2):
    _, outs = run_device(x, W1, b1, W2, b2, trace=False)
    return outs


# revision 9
# speedup vs baseline: 1.1012x; 1.0596x over previous
"""Channel-attention (mean + top-4 sum -> shared MLP -> sigmoid gate -> scale)
distributed over 8 TRN2 NeuronCores.

Layout: (b, c) on the 128 SBUF partitions, spatial on the free axis.
Sharding: the D spatial axis is split 8 ways (one contiguous chunk per core).
Per core: stream the 64 MiB local shard once, computing the per-(b,c) running
sum on ScalarE (activation accum_out) and the per-tile top-8 on VectorE
(InstMax) in the same pass; AllGather the tiny [128, 9] per-core stats; merge
(exact top-4 = top-4 of the 8 gathered descending top-8 lists); run the tiny
MLP on TensorE with block-diagonal weights (both batches in one matmul, both
branch stats as the 2-wide moving operand); then stream the shard a second
time multiplying by the per-partition sigmoid gate.
"""

import os
import sys

import numpy as np


def _ensure_imports():
    try:
        import concourse.bass  # noqa: F401
        return
    except ImportError:
        pass
    for p in ("/root/.axon_site/_ro/trn_rl_repo", "/opt/trn_rl_repo"):
        if os.path.isdir(p) and p not in sys.path:
            sys.path.append(p)
    import concourse.bass  # noqa: F401


_ensure_imports()

from concourse import bacc, mybir, tile  # noqa: E402
from concourse.bass_utils import run_bass_kernel_spmd  # noqa: E402

B, C, D, H, W = 2, 64, 64, 128, 128
NCORES = 8
P = B * C                # 128 partitions = (b, c)
DSH = D // NCORES        # 8 D-planes per core
F = DSH * H * W          # 131072 free elements per partition per core
FT = 8192                # free-dim tile size
NT = F // FT             # 16 tiles per pass
TOPK = 4
NCACHE = 6               # leading tiles kept resident in SBUF as bf16
F32 = mybir.dt.float32
BF16 = mybir.dt.bfloat16

_CACHE = {}


def _build():
    nc = bacc.Bacc(
        "TRN2", target_bir_lowering=False, debug=False, num_devices=NCORES
    )
    x_in = nc.declare_dram_parameter("x", [P, F], F32, isOutput=False)
    w1_in = nc.declare_dram_parameter("W1", [C // 2, C], F32, isOutput=False)
    b1_in = nc.declare_dram_parameter("b1", [1, C // 2], F32, isOutput=False)
    w2_in = nc.declare_dram_parameter("W2", [C, C // 2], F32, isOutput=False)
    b2_in = nc.declare_dram_parameter("b2", [1, C], F32, isOutput=False)
    out_x = nc.declare_dram_parameter("out", [P, F], F32, isOutput=True)
    gate_out = nc.declare_dram_parameter("gate", [P, 1], F32, isOutput=True)

    cc_in = nc.dram_tensor("cc_in", [P, 9], F32)
    cc_out = nc.dram_tensor("cc_out", [NCORES, P, 9], F32, addr_space="Shared")

    AFT = mybir.ActivationFunctionType
    AX = mybir.AxisListType
    rg = [list(range(NCORES))]
    HC = C // 2

    with tile.TileContext(nc) as tc:
        with (
            tc.tile_pool(name="big", bufs=3) as big,
            tc.tile_pool(name="cache", bufs=1) as cache,
            tc.tile_pool(name="small", bufs=1) as small,
            tc.tile_pool(name="ps", bufs=1, space="PSUM") as ps,
        ):
            # Block-diagonal MLP weights: both batches share the MLP, so one
            # [128,64] stationary computes fc1 for b=0 and b=1 at once.
            w1bd = small.tile([P, C], F32)       # lhsT: [k=bc, m=b*32+j]
            w2bd = small.tile([C, P], F32)       # lhsT: [k=b*32+j, m=bc]
            b1bd = small.tile([C, 1], F32)
            b2x2 = small.tile([P, 1], F32)       # 2*b2 (b2 appears in both fc branches)
            nc.vector.memset(w1bd[:], 0.0)
            nc.vector.memset(w2bd[:], 0.0)
            for b in range(B):
                nc.sync.dma_start(
                    out=w1bd[b * C:(b + 1) * C, b * HC:(b + 1) * HC],
                    in_=w1_in[:].rearrange("j c -> c j"),
                )
                nc.sync.dma_start(
                    out=w2bd[b * HC:(b + 1) * HC, b * C:(b + 1) * C],
                    in_=w2_in[:].rearrange("c j -> j c"),
                )
                nc.sync.dma_start(
                    out=b1bd[b * HC:(b + 1) * HC, :],
                    in_=b1_in[:].rearrange("a j -> j a"),
                )
                nc.sync.dma_start(
                    out=b2x2[b * C:(b + 1) * C, :],
                    in_=b2_in[:].rearrange("a c -> c a"),
                )
            nc.scalar.mul(b2x2[:], b2x2[:], 2.0)

            # Pass 1: stream the shard; ScalarE accumulates the per-tile sum
            # while VectorE extracts the per-tile top-8. The first NCACHE
            # tiles stay resident in SBUF as bf16 (cast fused into the
            # ScalarE sum-copy) so pass 2 can skip their HBM re-read.
            sum_slots = small.tile([P, NT], F32)
            top8_slots = small.tile([P, NT * 8], F32)
            cached = [
                cache.tile([P, FT], BF16, tag=f"cache{i}", name=f"cache{i}")
                for i in range(NCACHE)
            ]
            for i in range(NT):
                xt = big.tile([P, FT], F32, tag="xt")
                nc.sync.dma_start(out=xt[:], in_=x_in[:, i * FT:(i + 1) * FT])
                acc_dst = cached[i][:] if i < NCACHE else xt[:]
                nc.scalar.activation(
                    acc_dst, xt[:], AFT.Copy, accum_out=sum_slots[:, i:i + 1]
                )
                nc.vector.max(out=top8_slots[:, i * 8:(i + 1) * 8], in_=xt[:])

            # Local merge -> [top8 | sum] = [128, 9]
            stats9 = small.tile([P, 9], F32)
            nc.vector.max(out=stats9[:, 0:8], in_=top8_slots[:])
            nc.vector.reduce_sum(out=stats9[:, 8:9], in_=sum_slots[:], axis=AX.X)

            # Cross-core merge: tiny AllGather (8 x 4.5 KiB)
            nc.sync.dma_start(out=cc_in[:], in_=stats9[:])
            nc.gpsimd.collective_compute(
                "AllGather",
                mybir.AluOpType.bypass,
                replica_groups=rg,
                ins=[cc_in[:].opt()],
                outs=[cc_out[:].opt()],
            )
            gath = small.tile([P, NCORES * 9], F32)
            nc.sync.dma_start(
                out=gath[:].rearrange("p (r e) -> p r e", e=9),
                in_=cc_out[:].rearrange("r p e -> p r e"),
            )

            # Global merge: exact top-4 = first 4 of top-8 of the gathered
            # per-core descending top-8 lists; mean from the summed sums.
            g_top8 = small.tile([P, 8], F32)
            gv = gath[:].rearrange("p (r e) -> p r e", e=9)
            nc.vector.max(out=g_top8[:], in_=gv[:, :, 0:8])
            stats2 = small.tile([P, 2], F32)
            gs = gath[:].rearrange("p (r e) -> p e r", e=9)
            nc.vector.reduce_sum(out=stats2[:, 0:1], in_=gs[:, 8:9, :], axis=AX.X)
            nc.scalar.mul(stats2[:, 0:1], stats2[:, 0:1], 1.0 / (D * H * W))
            nc.vector.reduce_sum(out=stats2[:, 1:2], in_=g_top8[:, 0:4], axis=AX.X)

            # MLP: h = relu(W1 @ stats + b1); s = W2 @ h; gate = sigmoid(s0 + s1 + 2*b2)
            h_ps = ps.tile([C, B], F32)
            nc.tensor.matmul(h_ps[:], w1bd[:], stats2[:], start=True, stop=True)
            h_sb = small.tile([C, B], F32)
            nc.scalar.activation(h_sb[:], h_ps[:], AFT.Relu, bias=b1bd[:], scale=1.0)
            s_ps = ps.tile([P, B], F32)
            nc.tensor.matmul(s_ps[:], w2bd[:], h_sb[:], start=True, stop=True)
            ssum = small.tile([P, 1], F32)
            nc.vector.reduce_sum(out=ssum[:], in_=s_ps[:], axis=AX.X)
            gate_sb = small.tile([P, 1], F32)
            nc.scalar.activation(gate_sb[:], ssum[:], AFT.Sigmoid, bias=b2x2[:], scale=1.0)
            nc.scalar.dma_start(out=gate_out[:], in_=gate_sb[:])

            # Pass 2: scale by the per-partition gate. Streamed tiles first so
            # their loads prefetch through the gate bubble (loads on the Sync
            # HWDGE queue, stores on the Scalar HWDGE queue). Cached tiles
            # multiply in place (bf16, DVE 4x) and store with a cast-on-DMA
            # via SWDGE -- no staging tile, so no slot contention.
            for i in range(NCACHE, NT):
                yt = big.tile([P, FT], F32, tag="xt")
                nc.sync.dma_start(out=yt[:], in_=x_in[:, i * FT:(i + 1) * FT])
                nc.vector.tensor_scalar_mul(yt[:], yt[:], gate_sb[:])
                nc.scalar.dma_start(out=out_x[:, i * FT:(i + 1) * FT], in_=yt[:])
            for i in range(NCACHE):
                nc.vector.tensor_scalar_mul(cached[i][:], cached[i][:], gate_sb[:])
                nc.gpsimd.dma_start(out=out_x[:, i * FT:(i + 1) * FT], in_=cached[i][:])

    nc.compile()
    return nc


def _get_nc():
    if "nc" not in _CACHE:
        _CACHE["nc"] = _build()
    return _CACHE["nc"]


def _make_in_maps(x, W1, b1, W2, b2):
    x = np.ascontiguousarray(x, dtype=np.float32)
    w1 = np.ascontiguousarray(W1, dtype=np.float32)
    b1r = np.ascontiguousarray(b1, dtype=np.float32).reshape(1, HC_CONST)
    w2 = np.ascontiguousarray(W2, dtype=np.float32)
    b2r = np.ascontiguousarray(b2, dtype=np.float32).reshape(1, C)
    in_maps = []
    for i in range(NCORES):
        shard = np.ascontiguousarray(
            x[:, :, i * DSH:(i + 1) * DSH]
        ).reshape(P, F)
        in_maps.append(
            {"x": shard, "W1": w1, "b1": b1r, "W2": w2, "b2": b2r}
        )
    return in_maps


HC_CONST = C // 2


def run_device(x, W1, b1, W2, b2, trace=False, **kwargs):
    """Run the SPMD kernel; returns (BassKernelResults, assembled outputs)."""
    nc = _get_nc()
    in_maps = _make_in_maps(x, W1, b1, W2, b2)
    res = run_bass_kernel_spmd(
        nc, in_maps, core_ids=list(range(NCORES)), trace=trace, **kwargs
    )
    scaled = np.empty((B, C, D, H, W), dtype=np.float32)
    for i in range(NCORES):
        scaled[:, :, i * DSH:(i + 1) * DSH] = (
            res.results[i]["out"].reshape(B, C, DSH, H, W)
        )
    gate = res.results[0]["gate"].reshape(B, C)
    return res, (scaled, gate)


def kernel(x, W1, b1, W2, b2):
    _, outs = run_device(x, W1, b1, W2, b2, trace=False)
    return outs
